# revision 25
# baseline (speedup 1.0000x reference)
"""BiCrossAttention Trainium2 kernel.

Shards the (B=2, H=8) problem across 8 NeuronCores as (batch, head-pair):
core c handles batch c//4 and heads {2*(c%4), 2*(c%4)+1}.  Each core
computes its two heads' QKV projections, both cross-attention branches,
and a partial output projection; the host sums the 4 per-batch partials
and adds the bias.

Device-side schedule (flat, software-pipelined):
  - 8 "sweeps" = (pair, i-slab); pair p=0 couples (branch0,head0) with
    (branch1,head1) so their K=64 score matmuls dual-issue on disjoint
    PE row groups; p=1 couples the other diagonal.
  - per step (one 128-key j-chunk): score pair -> Exp (scalar engine,
    both members in one ACT) -> attnV for jc-2 (delayed so the tensor
    queue never stalls waiting on the Exp).
  - the softmax normalize of sweep w-1 is injected into the first ~6
    steps of sweep w so the slab boundary never bubbles the PE; the
    denominator comes free from an all-ones column appended to V.
  - k/v projections + V transposes are deadline-scheduled filler inside
    sweep 0; later sweeps carry only the next slab's q projections and
    the output-projection chunks, keeping the PE dense while the Exp
    cadence (~1us per j-chunk) gates the steady state.
  - DMA order front-loads the first-score dependencies (score weights,
    first x/context slabs in half-slab pieces).
  - alpha gating is folded into the V weights on the host; the partial
    output is returned in bf16 to halve the output DMA.
"""

import sys
import types

import numpy as np

for _p in ("/opt/trn_rl_repo",):
    if _p not in sys.path:
        sys.path.append(_p)

# Register the axon NTFF profile hook if the image's antenv lacks it (needed
# only when tracing; harmless otherwise).
try:
    import antenv

    if "antenv.axon_hooks" not in sys.modules:
        try:
            import antenv.axon_hooks  # noqa: F401
        except ImportError:
            _hooks = types.ModuleType("antenv.axon_hooks")
            _hook_holder = [None]
            _hooks.set_axon_ntff_profile_hook = lambda h: _hook_holder.__setitem__(0, h)
            _hooks.get_axon_ntff_profile_hook = lambda: _hook_holder[0]
            sys.modules["antenv.axon_hooks"] = _hooks
            antenv.axon_hooks = _hooks
            try:
                from trn_agent_boot.trn_boot import _ntff_profile_via_ctypes

                _hooks.set_axon_ntff_profile_hook(
                    _ntff_profile_via_ctypes("/opt/axon/libaxon_pjrt.so")
                )
            except Exception:
                pass
except Exception:
    pass

import ml_dtypes
import concourse.bacc as bacc
import concourse.mybir as mybir
import concourse.tile as tile
from concourse import bass_utils
from concourse.masks import make_identity

F32 = mybir.dt.float32
F32R = mybir.dt.float32r
BF16 = mybir.dt.bfloat16

_NP = {F32: np.float32, F32R: np.float32, BF16: ml_dtypes.bfloat16}

# Full problem constants
B, N, QD, CD, H, DH = 2, 2048, 1024, 1024, 8, 64
INNER = H * DH
SCALE = DH**-0.5
N_CORES = 8
HG = 4  # head-groups (of 2 heads) per batch


class Cfg:
    def __init__(self, n=N, d=QD, dt_proj=BF16, dt_attn=BF16, dt_out=BF16):
        self.N = n          # sequence length
        self.D = d          # model dim (= QD = CD)
        self.KC = d // 128  # contraction chunks for projections
        self.ISLAB = min(512, n)   # attention i-slab / projection i-chunk
        self.NJC = n // 128  # j chunks (128 keys each)
        self.dt_proj = dt_proj
        self.dt_attn = dt_attn
        self.dt_out = dt_out


def build_nc(cfg: Cfg):
    """Builds the single-core program (SPMD across all 8 cores)."""
    nc = bacc.Bacc("TRN2", target_bir_lowering=False, debug=False)
    KC, Nn, D = cfg.KC, cfg.N, cfg.D
    ISLAB, NJC = cfg.ISLAB, cfg.NJC
    NSL = Nn // ISLAB
    NCH = Nn // ISLAB
    JPC = ISLAB // 128  # j-chunks per 512-chunk
    DTP, DTA, DTO = cfg.dt_proj, cfg.dt_attn, cfg.dt_out

    xT = nc.dram_tensor(
        "xT", [NCH, 128, KC, ISLAB], DTP, kind="ExternalInput"
    ).ap()
    cT = nc.dram_tensor(
        "cT", [NCH, 128, KC, ISLAB], DTP, kind="ExternalInput"
    ).ap()
    wd = {
        name: nc.dram_tensor(name, [128, KC, 128], DTP, kind="ExternalInput").ap()
        for name in ("wq1", "wk1", "wv1", "wq2", "wk2", "wv2")
    }
    wout_d = nc.dram_tensor("wout", [128, D], DTO, kind="ExternalInput").ap()
    y_d = nc.dram_tensor("y", [Nn, D], DTO, kind="ExternalOutput").ap()

    SRC = {"1": xT, "2": cT}

    with tile.TileContext(nc) as tc:
        with (
            tc.tile_pool(name="const", bufs=1) as cpool,
            tc.tile_pool(name="qkv", bufs=1) as qkvpool,
            tc.tile_pool(name="vaug", bufs=1) as vaugpool,
            tc.tile_pool(name="outp", bufs=1) as outpool,
            tc.tile_pool(name="slab", bufs=8) as slabpool,
            tc.tile_pool(name="exp", bufs=6) as exppool,
            tc.tile_pool(name="tmp", bufs=6) as tmppool,
            tc.tile_pool(name="ysb", bufs=3) as ypool,
            tc.tile_pool(name="sim", bufs=2, space="PSUM") as simpool,
            tc.tile_pool(name="acc", bufs=2, space="PSUM") as accpool,
            tc.tile_pool(name="util", bufs=2, space="PSUM") as utilpool,
        ):
            # ---- DMA: front-load first-score dependencies ----
            w_sb = {}

            def dma_w(name):
                w_sb[name] = cpool.tile([128, KC, 128], DTP, tag=name, name=name)
                nc.sync.dma_start(out=w_sb[name][:], in_=wd[name])

            xs_tiles = {}

            def alloc_xs(stream, ch):
                t = slabpool.tile(
                    [128, KC, ISLAB], DTP, tag="xs", name=f"xs{stream}{ch}"
                )
                xs_tiles[(stream, ch)] = t
                return t

            xs20 = alloc_xs("2", 0)
            xs10 = alloc_xs("1", 0)
            # dual-queue ramp: weights stream on the Scalar HW-DGE queue
            # while the first x/context slab streams on Sync, so the
            # prologue projections start as soon as their operands land
            for name in ("wq2", "wk1", "wq1", "wk2", "wv1", "wv2"):
                w_sb[name] = cpool.tile([128, KC, 128], DTP, tag=name, name=name)
                nc.scalar.dma_start(out=w_sb[name][:], in_=wd[name])
            Q = KC // 4
            nc.sync.dma_start(out=xs10[:, 0:Q, :], in_=xT[0][:, 0:Q, :])
            nc.sync.dma_start(out=xs10[:, Q : 2 * Q, :], in_=xT[0][:, Q : 2 * Q, :])
            nc.sync.dma_start(out=xs20[:, 0 : KC // 2, :], in_=cT[0][:, 0 : KC // 2, :])
            nc.sync.dma_start(out=xs10[:, KC // 2 :, :], in_=xT[0][:, KC // 2 :, :])
            nc.sync.dma_start(out=xs20[:, KC // 2 :, :], in_=cT[0][:, KC // 2 :, :])
            for ch in range(1, NCH):
                for stream in ("2", "1"):
                    xs = alloc_xs(stream, ch)
                    nc.sync.dma_start(out=xs[:], in_=SRC[stream][ch])
            wout_sb = cpool.tile([128, D], DTO, tag="wout")
            nc.sync.dma_start(out=wout_sb[:], in_=wout_d)

            # ---- constants ----
            ident_f32 = cpool.tile([128, 128], F32, tag="ident_f32")
            make_identity(nc, ident_f32[:])
            ident = cpool.tile([128, 128], DTA, tag="ident")
            nc.vector.tensor_copy(out=ident[:], in_=ident_f32[:])
            ones_f32 = cpool.tile([128, 64], F32, tag="ones_f32")
            nc.vector.memset(ones_f32[:], 1.0)
            ones_sb = cpool.tile([128, 64], DTA, tag="ones")
            nc.vector.tensor_copy(out=ones_sb[:], in_=ones_f32[:])
            # dummy Exp: pulls the ~2.7us ACT table load under the DMA wait.
            # Writes into ones_f32[0, 1], which nothing consumes (the vaug
            # ones-column reads col 0; the bc broadcast reads row 64).
            nc.scalar.activation(
                ones_f32[0:1, 1:2], ones_f32[0:1, 0:1],
                mybir.ActivationFunctionType.Exp, scale=SCALE,
            )

            proj = {}
            for name in ("q1", "k1", "v1", "q2", "k2", "v2"):
                proj[name] = qkvpool.tile([128, Nn], DTA, tag=name, name=name)

            vaug = {}
            for br, h in ((0, 0), (0, 1), (1, 0), (1, 1)):
                vaug[(br, h)] = vaugpool.tile(
                    [128, NJC, 65], DTA, tag=f"vaug{br}{h}", name=f"vaug{br}{h}"
                )
                nc.vector.tensor_copy(
                    out=vaug[(br, h)][:, :, 64],
                    in_=ones_f32[:, 0:1].to_broadcast((128, NJC)),
                )

            # outT rows 0..63 = head 0 (written directly); head 1 staged in
            # outTB then DMA-shifted into rows 64..127 (compute engines are
            # lane-aligned and cannot move data across partitions; DMA can).
            outT = outpool.tile([128, Nn], DTO, tag="outT")
            outTB = outpool.tile([64, Nn], DTO, tag="outTB")

            # ---- emission helpers ----
            def proj_mms(pname, ch, kcs, pp):
                """Some kc contraction steps of projection chunk ch."""
                stream = pname[1]
                xs = xs_tiles[(stream, ch)]
                wt = w_sb["w" + pname]
                for kc in kcs:
                    nc.tensor.matmul(
                        pp[:], wt[:, kc, :], xs[:, kc, :],
                        start=(kc == 0), stop=(kc == KC - 1),
                    )

            def proj_finish(pname, ch, pp):
                nc.vector.tensor_copy(
                    out=proj[pname][:, ch * ISLAB : (ch + 1) * ISLAB], in_=pp[:]
                )

            def proj_chunk(pname, ch):
                """Full projection chunk (8 matmuls + copy-out), one util slot."""
                pp = utilpool.tile([128, ISLAB], F32, tag="util", name=f"pp{pname}{ch}")
                proj_mms(pname, ch, range(KC), pp)
                proj_finish(pname, ch, pp)

            def vaug_transp(br, ch):
                """Transpose chunk ch of v{br+1} into the vaug tiles."""
                vt = proj["v1" if br == 0 else "v2"]
                for jc in range(ch * JPC, (ch + 1) * JPC):
                    pt = utilpool.tile([128, 128], DTA, tag="util", name=f"pt{br}{jc}")
                    nc.tensor.transpose(
                        pt[:], vt[:, jc * 128 : (jc + 1) * 128], ident[:]
                    )
                    nc.vector.tensor_copy(out=vaug[(br, 0)][:, jc, 0:64], in_=pt[:, 0:64])
                    nc.vector.tensor_copy(out=vaug[(br, 1)][:, jc, 0:64], in_=pt[:, 64:128])

            # pair p=0: (br0, head0) + (br1, head1); p=1: (br0, head1) + (br1, head0)
            PAIRS = (((0, 0), (1, 1)), ((0, 1), (1, 0)))
            SWEEPS = [(p, sl) for p in (0, 1) for sl in range(NSL)]
            NSW = len(SWEEPS)

            exp_tiles = {}   # (w, jc) -> expP tile
            accs = {}        # w -> {(br,h): acc tile}

            def emit_scores(w, jc):
                p, sl = SWEEPS[w]
                i0 = sl * ISLAB
                jsl = slice(jc * 128, (jc + 1) * 128)
                simP = simpool.tile([128, 2, ISLAB], F32, tag="sim")
                for m, (br, h) in enumerate(PAIRS[p]):
                    q = proj["q2"] if br == 0 else proj["q1"]
                    k = proj["k1"] if br == 0 else proj["k2"]
                    rs = slice(h * 64, h * 64 + 64)
                    nc.tensor.matmul(
                        simP[:, m, :], k[rs, jsl], q[rs, i0 : i0 + ISLAB],
                        start=True, stop=True, tile_position=(h * 64, 0),
                    )
                expP = exppool.tile([128, 2, ISLAB], DTA, tag="exp")
                nc.scalar.activation(
                    expP[:], simP[:],
                    mybir.ActivationFunctionType.Exp, scale=SCALE,
                )
                exp_tiles[(w, jc)] = expP

            def emit_attnv(w, jc):
                p, sl = SWEEPS[w]
                if w not in accs:
                    accs[w] = {}
                    for br, h in PAIRS[p]:
                        accs[w][(br, h)] = accpool.tile(
                            [128, ISLAB], F32, tag="acc", name=f"acc{w}{br}{h}"
                        )
                expP = exp_tiles.pop((w, jc))
                for m, (br, h) in enumerate(PAIRS[p]):
                    nc.tensor.matmul(
                        accs[w][(br, h)][0:65, :], vaug[(br, h)][:, jc, :],
                        expP[:, m, :],
                        start=(jc == 0), stop=(jc == NJC - 1),
                    )

            norm_state = {}  # w -> dict with per-member tiles

            def emit_norm_a(w):
                """Reciprocal + value pull-out: frees the acc PSUM slots."""
                p, sl = SWEEPS[w]
                isl_ = slice(sl * ISLAB, (sl + 1) * ISLAB)
                st = {}
                for m, (br, h) in enumerate(PAIRS[p]):
                    acc = accs[w][(br, h)]
                    ot = outT if h == 0 else outTB
                    dst = ot[0:64, isl_]
                    if p != 0:
                        tmp = tmppool.tile([64, ISLAB], F32, tag="tmp", name=f"t{w}{m}")
                        st[f"tmp{m}"] = tmp
                        dst = tmp[0:64, :]
                    rcpf = tmppool.tile([128, ISLAB], F32, tag="rcpf", name=f"rf{w}{m}")
                    # the custom-DVE reciprocal mishandles nonzero base
                    # partitions, so run on a base-0 slice covering row 64
                    nc.vector.reciprocal_approx_fast(
                        out=rcpf[0:65, :], in_=acc[0:65, :]
                    )
                    nc.vector.tensor_copy(out=dst, in_=acc[0:64, :])
                    rcp = tmppool.tile([128, ISLAB], DTA, tag="rcpr", name=f"rc{w}{m}")
                    nc.vector.tensor_copy(out=rcp[64:65, :], in_=rcpf[64:65, :])
                    st[f"dst{m}"] = dst
                    st[f"rcp{m}"] = rcp
                del accs[w]
                norm_state[w] = st

            def emit_norm_b(w):
                """Broadcast the reciprocal across partitions and apply."""
                p, sl = SWEEPS[w]
                isl_ = slice(sl * ISLAB, (sl + 1) * ISLAB)
                st = norm_state.pop(w)
                for m, (br, h) in enumerate(PAIRS[p]):
                    bc = utilpool.tile([128, ISLAB], F32, tag="util", name=f"bc{w}{m}")
                    nc.tensor.matmul(
                        bc[0:64, :], ones_sb[64:65, :], st[f"rcp{m}"][64:65, :],
                        start=True, stop=True,
                    )
                    dst = st[f"dst{m}"]
                    nc.vector.tensor_mul(out=dst, in0=dst, in1=bc[0:64, :])
                    if p != 0:
                        ot = outT if h == 0 else outTB
                        nc.vector.tensor_add(
                            out=ot[0:64, isl_], in0=ot[0:64, isl_], in1=dst
                        )
                if p == 1:
                    # both branches final for this slab: shift head 1 into
                    # outT rows 64..127
                    nc.sync.dma_start(out=outT[64:128, isl_], in_=outTB[0:64, isl_])

            OCW = min(512, D)
            NOCS = D // OCW

            def emit_outproj(sl, ic, oc, scalar_cast=False, sim_slot=False):
                icsl = slice(ic * 128, (ic + 1) * 128)
                ocsl = slice(oc * OCW, (oc + 1) * OCW)
                if sim_slot:
                    # epilogue: the score PSUM banks are dead, reuse them
                    pyt = simpool.tile([128, 2, ISLAB], F32, tag="sim")
                    py = pyt[:, 0, :]
                else:
                    pyt = utilpool.tile(
                        [128, ISLAB], F32, tag="util", name=f"py{ic}{oc}"
                    )
                    py = pyt[:]
                nc.tensor.matmul(
                    py[:, 0:OCW], outT[:, icsl], wout_sb[:, ocsl],
                    start=True, stop=True,
                )
                ysb = ypool.tile([128, OCW], DTO, tag="ysb", name=f"ysb{ic}{oc}")
                if scalar_cast:
                    # epilogue: Exp stream is done, the scalar engine is idle
                    nc.scalar.activation(
                        ysb[:], py[:, 0:OCW],
                        mybir.ActivationFunctionType.Copy,
                    )
                else:
                    nc.vector.tensor_copy(out=ysb[:], in_=py[:, 0:OCW])
                nc.sync.dma_start(out=y_d[icsl, ocsl], in_=ysb[:])

            # ---- prologue: first-score projections, interleaved in halves
            # matching the DMA arrival order (two pp accumulators live) ----
            ppk1 = utilpool.tile([128, ISLAB], F32, tag="util", name="ppk1p")
            ppq2 = utilpool.tile([128, ISLAB], F32, tag="util", name="ppq2p")


            proj_mms("k1", 0, range(0, KC // 2), ppk1)
            proj_mms("q2", 0, range(0, KC // 2), ppq2)
            proj_mms("k1", 0, range(KC // 2, KC), ppk1)
            proj_mms("q2", 0, range(KC // 2, KC), ppq2)
            proj_finish("k1", 0, ppk1)
            proj_finish("q2", 0, ppq2)
            ppq1 = utilpool.tile([128, ISLAB], F32, tag="util", name="ppq1p")
            ppk2 = utilpool.tile([128, ISLAB], F32, tag="util", name="ppk2p")
            proj_mms("q1", 0, range(0, KC // 2), ppq1)
            proj_mms("k2", 0, range(0, KC // 2), ppk2)
            proj_mms("q1", 0, range(KC // 2, KC), ppq1)
            proj_mms("k2", 0, range(KC // 2, KC), ppk2)
            proj_finish("q1", 0, ppq1)
            proj_finish("k2", 0, ppk2)

            # ---- filler schedules ----
            # sweep 0 carries all k/v chunk projections + V transposes,
            # deadline-ordered (k(c) by step 4c-1 for its scores, vaug(c)
            # by step 4c+2 for its attnV at step 4c+3); sweeps 1-2 carry
            # the next slab's q projections; sweeps 4-7 carry the output
            # projections of the slab normalized at their step 6.
            qpp_state = {}

            def qproj_quarter(pname, nsl_, q):
                key = (pname, nsl_)
                if q == 0:
                    qpp_state[key] = utilpool.tile(
                        [128, ISLAB], F32, tag="util", name=f"pq{pname}{nsl_}"
                    )
                ppq = qpp_state[key]
                proj_mms(pname, nsl_, range(q * 2, q * 2 + 2), ppq)
                if q == 3:
                    proj_finish(pname, nsl_, ppq)
                    del qpp_state[key]

            def filler(w, jc):
                p, sl = SWEEPS[w]
                if w == 0:
                    sched = {
                        0: [("proj", "v1", 0), ("proj", "v2", 0)],
                        1: [("vaug", 0, 0), ("vaug", 1, 0)],
                        2: [("proj", "k1", 1), ("proj", "k2", 1)],
                        3: [("proj", "v1", 1)],
                        4: [("proj", "v2", 1)],
                        5: [("vaug", 0, 1), ("vaug", 1, 1)],
                        6: [("proj", "k1", 2), ("proj", "k2", 2)],
                        7: [("proj", "v1", 2)],
                        8: [("proj", "v2", 2)],
                        9: [("vaug", 0, 2), ("vaug", 1, 2)],
                        10: [("proj", "k1", 3), ("proj", "k2", 3)],
                        11: [("proj", "v1", 3)],
                        12: [("proj", "v2", 3)],
                        13: [("vaug", 0, 3), ("vaug", 1, 3)],
                        14: [("proj", "q2", 1)],
                        15: [("proj", "q1", 1)],
                    }.get(jc, [])
                    for item in sched:
                        if item[0] == "proj":
                            proj_chunk(item[1], item[2])
                        else:
                            vaug_transp(item[1], item[2])
                elif w < NSL - 1:
                    # q projections for slab sl+1, two kc per step
                    # (steps 7-14: clear of the norm_b util allocations)
                    if 7 <= jc < 11:
                        qproj_quarter("q2", sl + 1, jc - 7)
                    elif 11 <= jc < 15:
                        qproj_quarter("q1", sl + 1, jc - 11)
                elif w > NSL:
                    # output projection for the slab normalized at step 6
                    psl = sl - 1
                    if 7 <= jc < 7 + JPC * NOCS:
                        t = jc - 7
                        emit_outproj(psl, psl * JPC + t // NOCS, t % NOCS)

            # ---- main flat schedule (attnV delayed 3 steps everywhere) ----
            for w in range(NSW):
                for jc in range(NJC):
                    emit_scores(w, jc)
                    if jc < 3:
                        if w > 0:
                            emit_attnv(w - 1, NJC - 3 + jc)
                            if jc == 2:
                                emit_norm_a(w - 1)
                    else:
                        emit_attnv(w, jc - 3)
                        if jc == 6 and w > 0:
                            emit_norm_b(w - 1)
                    filler(w, jc)

            # ---- epilogue: fast normalize (no value pull-out; the acc
            # slots die here anyway), then the last slab's outproj ----
            w = NSW - 1
            p, sl = SWEEPS[w]
            isl_ = slice(sl * ISLAB, (sl + 1) * ISLAB)
            for jc in range(NJC - 3, NJC):
                emit_attnv(w, jc)
            st = {}
            for m, (br, h) in enumerate(PAIRS[p]):
                acc = accs[w][(br, h)]
                rcpf = tmppool.tile([128, ISLAB], F32, tag="rcpf", name=f"erf{m}")
                nc.vector.reciprocal_approx_fast(out=rcpf[0:65, :], in_=acc[0:65, :])
                tmp = tmppool.tile([64, ISLAB], F32, tag="tmp", name=f"et{m}")
                # value pull-out on the idle scalar engine, off the DVE chain
                nc.scalar.activation(
                    tmp[:], acc[0:64, :], mybir.ActivationFunctionType.Copy
                )
                rcp = tmppool.tile([128, ISLAB], DTA, tag="rcpr", name=f"erc{m}")
                nc.vector.tensor_copy(out=rcp[64:65, :], in_=rcpf[64:65, :])
                st[m] = (tmp, rcp)
            bcs = {}
            for m, (br, h) in enumerate(PAIRS[p]):
                bcs[m] = utilpool.tile([128, ISLAB], F32, tag="util", name=f"ebc{m}")
            for m, (br, h) in enumerate(PAIRS[p]):
                tmp, rcp = st[m]
                ot = outT if h == 0 else outTB
                bc = bcs[m]
                nc.tensor.matmul(
                    bc[0:64, :], ones_sb[64:65, :], rcp[64:65, :],
                    start=True, stop=True,
                )
                nc.vector.tensor_mul(out=tmp[:], in0=tmp[:], in1=bc[0:64, :])
                nc.vector.tensor_add(
                    out=ot[0:64, isl_], in0=ot[0:64, isl_], in1=tmp[:]
                )
            nc.scalar.dma_start(out=outT[64:128, isl_], in_=outTB[0:64, isl_])
            for t_, (ic, oc) in enumerate(
                (ic, oc)
                for ic in range((NSL - 1) * JPC, NSL * JPC)
                for oc in range(NOCS)
            ):
                emit_outproj(
                    NSL - 1, ic, oc,
                    scalar_cast=(t_ % 2 == 0), sim_slot=(t_ % 2 == 1),
                )

    nc.compile()
    return nc


_CACHE = {}
_ACTIVE_CFG = Cfg()


def _get_nc():
    if "nc" not in _CACHE:
        _CACHE["nc"] = build_nc(_ACTIVE_CFG)
    return _CACHE["nc"]


def _tile_kpart(a, dt):
    """[K, M] -> [128, K//128, M] with element (p, kc, m) = a[kc*128+p, m]."""
    k, m = a.shape
    return np.ascontiguousarray(
        a.reshape(k // 128, 128, m).transpose(1, 0, 2)
    ).astype(_NP[dt])


def make_in_maps(x, context, Wq1, Wk1, Wv1, Wq2, Wk2, Wv2, alpha_attn, Wout, bout):
    cfg = _ACTIVE_CFG
    alpha = float(1.0 / (1.0 + np.exp(-np.float64(alpha_attn))))
    Wv1s = np.asarray(Wv1, np.float32) * np.float32(alpha)
    Wv2s = np.asarray(Wv2, np.float32) * np.float32(1.0 - alpha)

    def _chunked(a):
        t = _tile_kpart(a, cfg.dt_proj)  # [128, KC, N]
        w = min(512, cfg.N)
        return np.ascontiguousarray(
            t.reshape(128, cfg.KC, cfg.N // w, w).transpose(2, 0, 1, 3)
        )

    xT = [_chunked(np.asarray(x[b], np.float32).T) for b in range(B)]
    cT = [_chunked(np.asarray(context[b], np.float32).T) for b in range(B)]

    in_maps = []
    for c in range(N_CORES):
        b, hg = c // HG, c % HG
        cols = slice(hg * 128, (hg + 1) * 128)
        in_maps.append(
            {
                "xT": xT[b],
                "cT": cT[b],
                "wq1": _tile_kpart(np.asarray(Wq1, np.float32)[:, cols], cfg.dt_proj),
                "wk1": _tile_kpart(np.asarray(Wk1, np.float32)[:, cols], cfg.dt_proj),
                "wv1": _tile_kpart(Wv1s[:, cols], cfg.dt_proj),
                "wq2": _tile_kpart(np.asarray(Wq2, np.float32)[:, cols], cfg.dt_proj),
                "wk2": _tile_kpart(np.asarray(Wk2, np.float32)[:, cols], cfg.dt_proj),
                "wv2": _tile_kpart(Wv2s[:, cols], cfg.dt_proj),
                "wout": np.ascontiguousarray(
                    np.asarray(Wout, np.float32)[cols, :]
                ).astype(_NP[cfg.dt_out]),
            }
        )
    return in_maps


def run_device(in_maps, trace=False, tmpdir=None):
    nc = _get_nc()
    return bass_utils.run_bass_kernel_spmd(
        nc, in_maps, core_ids=list(range(N_CORES)), trace=trace, tmpdir=tmpdir
    )


def kernel(x, context, Wq1, Wk1, Wv1, Wq2, Wk2, Wv2, alpha_attn, Wout, bout):
    in_maps = make_in_maps(
        x, context, Wq1, Wk1, Wv1, Wq2, Wk2, Wv2, alpha_attn, Wout, bout
    )
    res = run_device(in_maps)
    bout32 = np.asarray(bout, np.float32)
    out = np.empty((B, N, QD), np.float32)
    for b in range(B):
        acc = res.results[b * HG]["y"].astype(np.float32)
        for hg in range(1, HG):
            acc += res.results[b * HG + hg]["y"].astype(np.float32)
        out[b] = acc + bout32[None, :]
    return out


# revision 27
# speedup vs baseline: 1.0032x; 1.0032x over previous
"""BiCrossAttention Trainium2 kernel.

Shards the (B=2, H=8) problem across 8 NeuronCores as (batch, head-pair):
core c handles batch c//4 and heads {2*(c%4), 2*(c%4)+1}.  Each core
computes its two heads' QKV projections, both cross-attention branches,
and a partial output projection; the host sums the 4 per-batch partials
and adds the bias.

Device-side schedule (flat, software-pipelined):
  - 8 "sweeps" = (pair, i-slab); pair p=0 couples (branch0,head0) with
    (branch1,head1) so their K=64 score matmuls dual-issue on disjoint
    PE row groups; p=1 couples the other diagonal.
  - per step (one 128-key j-chunk): score pair -> Exp (scalar engine,
    both members in one ACT) -> attnV for jc-2 (delayed so the tensor
    queue never stalls waiting on the Exp).
  - the softmax normalize of sweep w-1 is injected into the first ~6
    steps of sweep w so the slab boundary never bubbles the PE; the
    denominator comes free from an all-ones column appended to V.
  - k/v projections + V transposes are deadline-scheduled filler inside
    sweep 0; later sweeps carry only the next slab's q projections and
    the output-projection chunks, keeping the PE dense while the Exp
    cadence (~1us per j-chunk) gates the steady state.
  - DMA order front-loads the first-score dependencies (score weights,
    first x/context slabs in half-slab pieces).
  - alpha gating is folded into the V weights on the host; the partial
    output is returned in bf16 to halve the output DMA.
"""

import sys
import types

import numpy as np

for _p in ("/opt/trn_rl_repo",):
    if _p not in sys.path:
        sys.path.append(_p)

# Register the axon NTFF profile hook if the image's antenv lacks it (needed
# only when tracing; harmless otherwise).
try:
    import antenv

    if "antenv.axon_hooks" not in sys.modules:
        try:
            import antenv.axon_hooks  # noqa: F401
        except ImportError:
            _hooks = types.ModuleType("antenv.axon_hooks")
            _hook_holder = [None]
            _hooks.set_axon_ntff_profile_hook = lambda h: _hook_holder.__setitem__(0, h)
            _hooks.get_axon_ntff_profile_hook = lambda: _hook_holder[0]
            sys.modules["antenv.axon_hooks"] = _hooks
            antenv.axon_hooks = _hooks
            try:
                from trn_agent_boot.trn_boot import _ntff_profile_via_ctypes

                _hooks.set_axon_ntff_profile_hook(
                    _ntff_profile_via_ctypes("/opt/axon/libaxon_pjrt.so")
                )
            except Exception:
                pass
except Exception:
    pass

import ml_dtypes
import concourse.bacc as bacc
import concourse.mybir as mybir
import concourse.tile as tile
from concourse import bass_utils
from concourse.masks import make_identity

F32 = mybir.dt.float32
F32R = mybir.dt.float32r
BF16 = mybir.dt.bfloat16

_NP = {F32: np.float32, F32R: np.float32, BF16: ml_dtypes.bfloat16}

# Full problem constants
B, N, QD, CD, H, DH = 2, 2048, 1024, 1024, 8, 64
INNER = H * DH
SCALE = DH**-0.5
N_CORES = 8
HG = 4  # head-groups (of 2 heads) per batch


class Cfg:
    def __init__(self, n=N, d=QD, dt_proj=BF16, dt_attn=BF16, dt_out=BF16):
        self.N = n          # sequence length
        self.D = d          # model dim (= QD = CD)
        self.KC = d // 128  # contraction chunks for projections
        self.ISLAB = min(512, n)   # attention i-slab / projection i-chunk
        self.NJC = n // 128  # j chunks (128 keys each)
        self.dt_proj = dt_proj
        self.dt_attn = dt_attn
        self.dt_out = dt_out


def build_nc(cfg: Cfg):
    """Builds the single-core program (SPMD across all 8 cores)."""
    nc = bacc.Bacc("TRN2", target_bir_lowering=False, debug=False)
    KC, Nn, D = cfg.KC, cfg.N, cfg.D
    ISLAB, NJC = cfg.ISLAB, cfg.NJC
    NSL = Nn // ISLAB
    NCH = Nn // ISLAB
    JPC = ISLAB // 128  # j-chunks per 512-chunk
    DTP, DTA, DTO = cfg.dt_proj, cfg.dt_attn, cfg.dt_out

    xT = nc.dram_tensor(
        "xT", [NCH, 128, KC, ISLAB], DTP, kind="ExternalInput"
    ).ap()
    cT = nc.dram_tensor(
        "cT", [NCH, 128, KC, ISLAB], DTP, kind="ExternalInput"
    ).ap()
    wd = {
        name: nc.dram_tensor(name, [128, KC, 128], DTP, kind="ExternalInput").ap()
        for name in ("wq1", "wk1", "wv1", "wq2", "wk2", "wv2")
    }
    wout_d = nc.dram_tensor("wout", [128, D], DTO, kind="ExternalInput").ap()
    y_d = nc.dram_tensor("y", [Nn, D], DTO, kind="ExternalOutput").ap()

    SRC = {"1": xT, "2": cT}

    with tile.TileContext(nc) as tc:
        with (
            tc.tile_pool(name="const", bufs=1) as cpool,
            tc.tile_pool(name="qkv", bufs=1) as qkvpool,
            tc.tile_pool(name="vaug", bufs=1) as vaugpool,
            tc.tile_pool(name="outp", bufs=1) as outpool,
            tc.tile_pool(name="slab", bufs=8) as slabpool,
            tc.tile_pool(name="exp", bufs=6) as exppool,
            tc.tile_pool(name="tmp", bufs=6) as tmppool,
            tc.tile_pool(name="ysb", bufs=3) as ypool,
            tc.tile_pool(name="sim", bufs=2, space="PSUM") as simpool,
            tc.tile_pool(name="acc", bufs=2, space="PSUM") as accpool,
            tc.tile_pool(name="util", bufs=2, space="PSUM") as utilpool,
        ):
            # ---- DMA: front-load first-score dependencies ----
            w_sb = {}

            def dma_w(name):
                w_sb[name] = cpool.tile([128, KC, 128], DTP, tag=name, name=name)
                nc.sync.dma_start(out=w_sb[name][:], in_=wd[name])

            xs_tiles = {}

            def alloc_xs(stream, ch):
                t = slabpool.tile(
                    [128, KC, ISLAB], DTP, tag="xs", name=f"xs{stream}{ch}"
                )
                xs_tiles[(stream, ch)] = t
                return t

            xs20 = alloc_xs("2", 0)
            xs10 = alloc_xs("1", 0)
            # dual-queue ramp: weights stream on the Scalar HW-DGE queue
            # while the first x/context slab streams on Sync, so the
            # prologue projections start as soon as their operands land
            for name in ("wk1", "wq2", "wq1", "wk2", "wv1", "wv2"):
                w_sb[name] = cpool.tile([128, KC, 128], DTP, tag=name, name=name)
                nc.scalar.dma_start(out=w_sb[name][:], in_=wd[name])
            Q = KC // 4
            nc.sync.dma_start(out=xs10[:, 0:Q, :], in_=xT[0][:, 0:Q, :])
            nc.sync.dma_start(out=xs10[:, Q : 2 * Q, :], in_=xT[0][:, Q : 2 * Q, :])
            nc.sync.dma_start(out=xs20[:, 0 : KC // 2, :], in_=cT[0][:, 0 : KC // 2, :])
            nc.sync.dma_start(out=xs10[:, KC // 2 :, :], in_=xT[0][:, KC // 2 :, :])
            nc.sync.dma_start(out=xs20[:, KC // 2 :, :], in_=cT[0][:, KC // 2 :, :])
            for ch in range(1, NCH):
                for stream in ("2", "1"):
                    xs = alloc_xs(stream, ch)
                    nc.sync.dma_start(out=xs[:], in_=SRC[stream][ch])
            wout_sb = cpool.tile([128, D], DTO, tag="wout")
            nc.sync.dma_start(out=wout_sb[:], in_=wout_d)

            # ---- constants ----
            ident_f32 = cpool.tile([128, 128], F32, tag="ident_f32")
            make_identity(nc, ident_f32[:])
            ident = cpool.tile([128, 128], DTA, tag="ident")
            nc.vector.tensor_copy(out=ident[:], in_=ident_f32[:])
            ones_f32 = cpool.tile([128, 64], F32, tag="ones_f32")
            nc.vector.memset(ones_f32[:], 1.0)
            ones_sb = cpool.tile([128, 64], DTA, tag="ones")
            nc.vector.tensor_copy(out=ones_sb[:], in_=ones_f32[:])
            # dummy Exp: pulls the ~2.7us ACT table load under the DMA wait.
            # Writes into ones_f32[0, 1], which nothing consumes (the vaug
            # ones-column reads col 0; the bc broadcast reads row 64).
            nc.scalar.activation(
                ones_f32[0:1, 1:2], ones_f32[0:1, 0:1],
                mybir.ActivationFunctionType.Exp, scale=SCALE,
            )

            proj = {}
            for name in ("q1", "k1", "v1", "q2", "k2", "v2"):
                proj[name] = qkvpool.tile([128, Nn], DTA, tag=name, name=name)

            vaug = {}
            for br, h in ((0, 0), (0, 1), (1, 0), (1, 1)):
                vaug[(br, h)] = vaugpool.tile(
                    [128, NJC, 65], DTA, tag=f"vaug{br}{h}", name=f"vaug{br}{h}"
                )
                nc.vector.tensor_copy(
                    out=vaug[(br, h)][:, :, 64],
                    in_=ones_f32[:, 0:1].to_broadcast((128, NJC)),
                )

            # outT rows 0..63 = head 0 (written directly); head 1 staged in
            # outTB then DMA-shifted into rows 64..127 (compute engines are
            # lane-aligned and cannot move data across partitions; DMA can).
            outT = outpool.tile([128, Nn], DTO, tag="outT")
            outTB = outpool.tile([64, Nn], DTO, tag="outTB")

            # ---- emission helpers ----
            def proj_mms(pname, ch, kcs, pp):
                """Some kc contraction steps of projection chunk ch."""
                stream = pname[1]
                xs = xs_tiles[(stream, ch)]
                wt = w_sb["w" + pname]
                for kc in kcs:
                    nc.tensor.matmul(
                        pp[:], wt[:, kc, :], xs[:, kc, :],
                        start=(kc == 0), stop=(kc == KC - 1),
                    )

            def proj_finish(pname, ch, pp):
                nc.vector.tensor_copy(
                    out=proj[pname][:, ch * ISLAB : (ch + 1) * ISLAB], in_=pp[:]
                )

            def proj_chunk(pname, ch):
                """Full projection chunk (8 matmuls + copy-out), one util slot."""
                pp = utilpool.tile([128, ISLAB], F32, tag="util", name=f"pp{pname}{ch}")
                proj_mms(pname, ch, range(KC), pp)
                proj_finish(pname, ch, pp)

            def vaug_transp(br, ch):
                """Transpose chunk ch of v{br+1} into the vaug tiles."""
                vt = proj["v1" if br == 0 else "v2"]
                for jc in range(ch * JPC, (ch + 1) * JPC):
                    pt = utilpool.tile([128, 128], DTA, tag="util", name=f"pt{br}{jc}")
                    nc.tensor.transpose(
                        pt[:], vt[:, jc * 128 : (jc + 1) * 128], ident[:]
                    )
                    nc.vector.tensor_copy(out=vaug[(br, 0)][:, jc, 0:64], in_=pt[:, 0:64])
                    nc.vector.tensor_copy(out=vaug[(br, 1)][:, jc, 0:64], in_=pt[:, 64:128])

            # pair p=0: (br0, head0) + (br1, head1); p=1: (br0, head1) + (br1, head0)
            PAIRS = (((0, 0), (1, 1)), ((0, 1), (1, 0)))
            SWEEPS = [(p, sl) for p in (0, 1) for sl in range(NSL)]
            NSW = len(SWEEPS)

            exp_tiles = {}   # (w, jc) -> expP tile
            accs = {}        # w -> {(br,h): acc tile}

            def emit_scores(w, jc):
                p, sl = SWEEPS[w]
                i0 = sl * ISLAB
                jsl = slice(jc * 128, (jc + 1) * 128)
                simP = simpool.tile([128, 2, ISLAB], F32, tag="sim")
                for m, (br, h) in enumerate(PAIRS[p]):
                    q = proj["q2"] if br == 0 else proj["q1"]
                    k = proj["k1"] if br == 0 else proj["k2"]
                    rs = slice(h * 64, h * 64 + 64)
                    nc.tensor.matmul(
                        simP[:, m, :], k[rs, jsl], q[rs, i0 : i0 + ISLAB],
                        start=True, stop=True, tile_position=(h * 64, 0),
                    )
                expP = exppool.tile([128, 2, ISLAB], DTA, tag="exp")
                nc.scalar.activation(
                    expP[:], simP[:],
                    mybir.ActivationFunctionType.Exp, scale=SCALE,
                )
                exp_tiles[(w, jc)] = expP

            def emit_attnv(w, jc):
                p, sl = SWEEPS[w]
                if w not in accs:
                    accs[w] = {}
                    for br, h in PAIRS[p]:
                        accs[w][(br, h)] = accpool.tile(
                            [128, ISLAB], F32, tag="acc", name=f"acc{w}{br}{h}"
                        )
                expP = exp_tiles.pop((w, jc))
                for m, (br, h) in enumerate(PAIRS[p]):
                    nc.tensor.matmul(
                        accs[w][(br, h)][0:65, :], vaug[(br, h)][:, jc, :],
                        expP[:, m, :],
                        start=(jc == 0), stop=(jc == NJC - 1),
                    )

            norm_state = {}  # w -> dict with per-member tiles

            def emit_norm_a(w):
                """Reciprocal + value pull-out: frees the acc PSUM slots."""
                p, sl = SWEEPS[w]
                isl_ = slice(sl * ISLAB, (sl + 1) * ISLAB)
                st = {}
                for m, (br, h) in enumerate(PAIRS[p]):
                    acc = accs[w][(br, h)]
                    ot = outT if h == 0 else outTB
                    dst = ot[0:64, isl_]
                    if p != 0:
                        tmp = tmppool.tile([64, ISLAB], F32, tag="tmp", name=f"t{w}{m}")
                        st[f"tmp{m}"] = tmp
                        dst = tmp[0:64, :]
                    rcpf = tmppool.tile([128, ISLAB], F32, tag="rcpf", name=f"rf{w}{m}")
                    # the custom-DVE reciprocal mishandles nonzero base
                    # partitions, so run on a base-0 slice covering row 64
                    nc.vector.reciprocal_approx_fast(
                        out=rcpf[0:65, :], in_=acc[0:65, :]
                    )
                    nc.vector.tensor_copy(out=dst, in_=acc[0:64, :])
                    rcp = tmppool.tile([128, ISLAB], DTA, tag="rcpr", name=f"rc{w}{m}")
                    nc.vector.tensor_copy(out=rcp[64:65, :], in_=rcpf[64:65, :])
                    st[f"dst{m}"] = dst
                    st[f"rcp{m}"] = rcp
                del accs[w]
                norm_state[w] = st

            def emit_norm_b(w):
                """Broadcast the reciprocal across partitions and apply."""
                p, sl = SWEEPS[w]
                isl_ = slice(sl * ISLAB, (sl + 1) * ISLAB)
                st = norm_state.pop(w)
                for m, (br, h) in enumerate(PAIRS[p]):
                    bc = utilpool.tile([128, ISLAB], F32, tag="util", name=f"bc{w}{m}")
                    nc.tensor.matmul(
                        bc[0:64, :], ones_sb[64:65, :], st[f"rcp{m}"][64:65, :],
                        start=True, stop=True,
                    )
                    dst = st[f"dst{m}"]
                    nc.vector.tensor_mul(out=dst, in0=dst, in1=bc[0:64, :])
                    if p != 0:
                        ot = outT if h == 0 else outTB
                        nc.vector.tensor_add(
                            out=ot[0:64, isl_], in0=ot[0:64, isl_], in1=dst
                        )
                if p == 1:
                    # both branches final for this slab: shift head 1 into
                    # outT rows 64..127
                    nc.sync.dma_start(out=outT[64:128, isl_], in_=outTB[0:64, isl_])

            OCW = min(512, D)
            NOCS = D // OCW

            def emit_outproj(sl, ic, oc, scalar_cast=False, sim_slot=False):
                icsl = slice(ic * 128, (ic + 1) * 128)
                ocsl = slice(oc * OCW, (oc + 1) * OCW)
                if sim_slot:
                    # epilogue: the score PSUM banks are dead, reuse them
                    pyt = simpool.tile([128, 2, ISLAB], F32, tag="sim")
                    py = pyt[:, 0, :]
                else:
                    pyt = utilpool.tile(
                        [128, ISLAB], F32, tag="util", name=f"py{ic}{oc}"
                    )
                    py = pyt[:]
                nc.tensor.matmul(
                    py[:, 0:OCW], outT[:, icsl], wout_sb[:, ocsl],
                    start=True, stop=True,
                )
                ysb = ypool.tile([128, OCW], DTO, tag="ysb", name=f"ysb{ic}{oc}")
                if scalar_cast:
                    # epilogue: Exp stream is done, the scalar engine is idle
                    nc.scalar.activation(
                        ysb[:], py[:, 0:OCW],
                        mybir.ActivationFunctionType.Copy,
                    )
                else:
                    nc.vector.tensor_copy(out=ysb[:], in_=py[:, 0:OCW])
                nc.sync.dma_start(out=y_d[icsl, ocsl], in_=ysb[:])

            # ---- prologue: first-score projections, interleaved in halves
            # matching the DMA arrival order (two pp accumulators live) ----
            ppk1 = utilpool.tile([128, ISLAB], F32, tag="util", name="ppk1p")
            ppq2 = utilpool.tile([128, ISLAB], F32, tag="util", name="ppq2p")


            proj_mms("k1", 0, range(0, KC // 2), ppk1)
            proj_mms("q2", 0, range(0, KC // 2), ppq2)
            proj_mms("k1", 0, range(KC // 2, KC), ppk1)
            proj_mms("q2", 0, range(KC // 2, KC), ppq2)
            proj_finish("k1", 0, ppk1)
            proj_finish("q2", 0, ppq2)
            ppq1 = utilpool.tile([128, ISLAB], F32, tag="util", name="ppq1p")
            ppk2 = utilpool.tile([128, ISLAB], F32, tag="util", name="ppk2p")
            proj_mms("q1", 0, range(0, KC // 2), ppq1)
            proj_mms("k2", 0, range(0, KC // 2), ppk2)
            proj_mms("q1", 0, range(KC // 2, KC), ppq1)
            proj_mms("k2", 0, range(KC // 2, KC), ppk2)
            proj_finish("q1", 0, ppq1)
            proj_finish("k2", 0, ppk2)

            # ---- filler schedules ----
            # sweep 0 carries all k/v chunk projections + V transposes,
            # deadline-ordered (k(c) by step 4c-1 for its scores, vaug(c)
            # by step 4c+2 for its attnV at step 4c+3); sweeps 1-2 carry
            # the next slab's q projections; sweeps 4-7 carry the output
            # projections of the slab normalized at their step 6.
            qpp_state = {}

            def qproj_quarter(pname, nsl_, q):
                key = (pname, nsl_)
                if q == 0:
                    qpp_state[key] = utilpool.tile(
                        [128, ISLAB], F32, tag="util", name=f"pq{pname}{nsl_}"
                    )
                ppq = qpp_state[key]
                proj_mms(pname, nsl_, range(q * 2, q * 2 + 2), ppq)
                if q == 3:
                    proj_finish(pname, nsl_, ppq)
                    del qpp_state[key]

            def filler(w, jc):
                p, sl = SWEEPS[w]
                if w == 0:
                    sched = {
                        0: [("proj", "v1", 0), ("proj", "v2", 0)],
                        1: [("vaug", 0, 0), ("vaug", 1, 0)],
                        2: [("proj", "k1", 1), ("proj", "k2", 1)],
                        3: [("proj", "v1", 1)],
                        4: [("proj", "v2", 1)],
                        5: [("vaug", 0, 1), ("vaug", 1, 1)],
                        6: [("proj", "k1", 2), ("proj", "k2", 2)],
                        7: [("proj", "v1", 2)],
                        8: [("proj", "v2", 2)],
                        9: [("vaug", 0, 2), ("vaug", 1, 2)],
                        10: [("proj", "k1", 3), ("proj", "k2", 3)],
                        11: [("proj", "v1", 3)],
                        12: [("proj", "v2", 3)],
                        13: [("vaug", 0, 3), ("vaug", 1, 3)],
                        14: [("proj", "q2", 1)],
                        15: [("proj", "q1", 1)],
                    }.get(jc, [])
                    for item in sched:
                        if item[0] == "proj":
                            proj_chunk(item[1], item[2])
                        else:
                            vaug_transp(item[1], item[2])
                elif w < NSL - 1:
                    # q projections for slab sl+1, two kc per step
                    # (steps 7-14: clear of the norm_b util allocations)
                    if 7 <= jc < 11:
                        qproj_quarter("q2", sl + 1, jc - 7)
                    elif 11 <= jc < 15:
                        qproj_quarter("q1", sl + 1, jc - 11)
                elif w > NSL:
                    # output projection for the slab normalized at step 6
                    psl = sl - 1
                    if 7 <= jc < 7 + JPC * NOCS:
                        t = jc - 7
                        emit_outproj(psl, psl * JPC + t // NOCS, t % NOCS)

            # ---- main flat schedule (attnV delayed 3 steps everywhere) ----
            for w in range(NSW):
                for jc in range(NJC):
                    emit_scores(w, jc)
                    if jc < 3:
                        if w > 0:
                            emit_attnv(w - 1, NJC - 3 + jc)
                            if jc == 2:
                                emit_norm_a(w - 1)
                    else:
                        emit_attnv(w, jc - 3)
                        if jc == 6 and w > 0:
                            emit_norm_b(w - 1)
                    filler(w, jc)

            # ---- epilogue: normalize + outproj in two column halves so
            # the first half's outproj overlaps the second half's chain ----
            w = NSW - 1
            p, sl = SWEEPS[w]
            for jc in range(NJC - 3, NJC):
                emit_attnv(w, jc)
            HWC = ISLAB // 2
            for half in range(2):
                hs = slice(half * HWC, (half + 1) * HWC)
                gsl = slice(sl * ISLAB + half * HWC, sl * ISLAB + (half + 1) * HWC)
                st = {}
                for m, (br, h) in enumerate(PAIRS[p]):
                    acc = accs[w][(br, h)]
                    rcpf = tmppool.tile(
                        [128, HWC], F32, tag="rcpf", name=f"erf{half}{m}"
                    )
                    nc.vector.reciprocal_approx_fast(
                        out=rcpf[0:65, :], in_=acc[0:65, hs]
                    )
                    tmp = tmppool.tile([64, HWC], F32, tag="tmp", name=f"et{half}{m}")
                    # value pull-out on the now-idle scalar engine
                    nc.scalar.activation(
                        tmp[:], acc[0:64, hs], mybir.ActivationFunctionType.Copy
                    )
                    rcp = tmppool.tile(
                        [128, HWC], DTA, tag="rcpr", name=f"erc{half}{m}"
                    )
                    nc.vector.tensor_copy(out=rcp[64:65, :], in_=rcpf[64:65, :])
                    st[m] = (tmp, rcp)
                for m, (br, h) in enumerate(PAIRS[p]):
                    tmp, rcp = st[m]
                    ot = outT if h == 0 else outTB
                    bc = utilpool.tile(
                        [128, ISLAB], F32, tag="util", name=f"ebc{half}{m}"
                    )
                    nc.tensor.matmul(
                        bc[0:64, 0:HWC], ones_sb[64:65, :], rcp[64:65, :],
                        start=True, stop=True,
                    )
                    nc.vector.tensor_mul(out=tmp[:], in0=tmp[:], in1=bc[0:64, 0:HWC])
                    nc.vector.tensor_add(
                        out=ot[0:64, gsl], in0=ot[0:64, gsl], in1=tmp[:]
                    )
                nc.scalar.dma_start(out=outT[64:128, gsl], in_=outTB[0:64, gsl])
                for t_, (icl, oc) in enumerate(
                    (icl, oc) for icl in range(JPC // 2) for oc in range(NOCS)
                ):
                    emit_outproj(
                        NSL - 1,
                        (NSL - 1) * JPC + half * (JPC // 2) + icl,
                        oc,
                        scalar_cast=(t_ % 2 == 0), sim_slot=(t_ % 2 == 1),
                    )

    nc.compile()
    return nc


_CACHE = {}
_ACTIVE_CFG = Cfg()


def _get_nc():
    if "nc" not in _CACHE:
        _CACHE["nc"] = build_nc(_ACTIVE_CFG)
    return _CACHE["nc"]


def _tile_kpart(a, dt):
    """[K, M] -> [128, K//128, M] with element (p, kc, m) = a[kc*128+p, m]."""
    k, m = a.shape
    return np.ascontiguousarray(
        a.reshape(k // 128, 128, m).transpose(1, 0, 2)
    ).astype(_NP[dt])


def make_in_maps(x, context, Wq1, Wk1, Wv1, Wq2, Wk2, Wv2, alpha_attn, Wout, bout):
    cfg = _ACTIVE_CFG
    alpha = float(1.0 / (1.0 + np.exp(-np.float64(alpha_attn))))
    Wv1s = np.asarray(Wv1, np.float32) * np.float32(alpha)
    Wv2s = np.asarray(Wv2, np.float32) * np.float32(1.0 - alpha)

    def _chunked(a):
        t = _tile_kpart(a, cfg.dt_proj)  # [128, KC, N]
        w = min(512, cfg.N)
        return np.ascontiguousarray(
            t.reshape(128, cfg.KC, cfg.N // w, w).transpose(2, 0, 1, 3)
        )

    xT = [_chunked(np.asarray(x[b], np.float32).T) for b in range(B)]
    cT = [_chunked(np.asarray(context[b], np.float32).T) for b in range(B)]

    in_maps = []
    for c in range(N_CORES):
        b, hg = c // HG, c % HG
        cols = slice(hg * 128, (hg + 1) * 128)
        in_maps.append(
            {
                "xT": xT[b],
                "cT": cT[b],
                "wq1": _tile_kpart(np.asarray(Wq1, np.float32)[:, cols], cfg.dt_proj),
                "wk1": _tile_kpart(np.asarray(Wk1, np.float32)[:, cols], cfg.dt_proj),
                "wv1": _tile_kpart(Wv1s[:, cols], cfg.dt_proj),
                "wq2": _tile_kpart(np.asarray(Wq2, np.float32)[:, cols], cfg.dt_proj),
                "wk2": _tile_kpart(np.asarray(Wk2, np.float32)[:, cols], cfg.dt_proj),
                "wv2": _tile_kpart(Wv2s[:, cols], cfg.dt_proj),
                "wout": np.ascontiguousarray(
                    np.asarray(Wout, np.float32)[cols, :]
                ).astype(_NP[cfg.dt_out]),
            }
        )
    return in_maps


def run_device(in_maps, trace=False, tmpdir=None):
    nc = _get_nc()
    return bass_utils.run_bass_kernel_spmd(
        nc, in_maps, core_ids=list(range(N_CORES)), trace=trace, tmpdir=tmpdir
    )


def kernel(x, context, Wq1, Wk1, Wv1, Wq2, Wk2, Wv2, alpha_attn, Wout, bout):
    in_maps = make_in_maps(
        x, context, Wq1, Wk1, Wv1, Wq2, Wk2, Wv2, alpha_attn, Wout, bout
    )
    res = run_device(in_maps)
    bout32 = np.asarray(bout, np.float32)
    out = np.empty((B, N, QD), np.float32)
    for b in range(B):
        acc = res.results[b * HG]["y"].astype(np.float32)
        for hg in range(1, HG):
            acc += res.results[b * HG + hg]["y"].astype(np.float32)
        out[b] = acc + bout32[None, :]
    return out
